# revision 33
# baseline (speedup 1.0000x reference)
"""Multi-head attention (B=4, S=2048, D=1024, H=16) on 8 TRN2 NeuronCores.

Sharding (no collectives): core c handles batch b = c//2 and sequence-half
h = c%2 (1024 of 2048 query rows), with ALL 16 heads local. K/V projections
are computed for the full sequence on every core (the only duplicated work);
Q projection, attention, and the output projection only cover the core's
sequence half. The program is identical on all cores (pure SPMD) -- the
host rotates each core's inputs along the sequence axis by h*1024 so the
core's own half always sits at columns [0, 1024).

Everything on-chip is computed in a transposed layout (feature dim on
partitions) so no on-chip transposes are needed:
  qT/kT [D, s]   scoresT [sk, sq]   outT [d, sq]   finalT [dcol, sq]
The host pre-transposes query/weights/mask (cheap numpy work, not HW time).

Softmax: reference masking is multiplicative (scores * mask), so masked
slots contribute exp(0)=1 to the softmax. We compute e = exp(s/8), patch
1.0 where mask==0 (one DVE copy_predicated per tile), and the denominator
comes free from the PV matmul via a ones-column appended to v (row 64 of
the [65, nb] PV result is the row-sum of attention weights).
"""

import os
import sys

import numpy as np

for _p in ("/opt/trn_rl_repo",):
    if _p not in sys.path and os.path.isdir(_p):
        sys.path.insert(0, _p)

import ml_dtypes

import concourse.bass as bass
from concourse import bacc
import concourse.mybir as mybir
import concourse.tile as tile
from concourse.bass_utils import run_bass_kernel_spmd

BF16 = mybir.dt.bfloat16
F32 = mybir.dt.float32
AF = mybir.ActivationFunctionType

B, S, D, H, DH = 4, 2048, 1024, 16, 64
NCORES = 8
SH = S // 2          # sequence half per core = 1024
P = 128              # partitions
NB = 512             # matmul free-dim block
KC = D // P          # 8 contraction chunks of 128
SKC = S // P         # 16 key chunks of 128
SKH = SKC // 2       # key chunks per half-pass = 8
SQB = SH // NB       # 2 query-col blocks of 512
NPAIR = H // 2       # 8 head pairs
VW = DH + 1          # 65: v columns + ones column
VROW = H * VW        # 1040: one sk-chunk row of packed v
NROW = H * SQB       # 32 softmax-denominator rows

_bf16 = ml_dtypes.bfloat16


def _build_bass():
    nc = bacc.Bacc(num_devices=NCORES)

    qT_d = nc.declare_dram_parameter("qT", [D, S], BF16, isOutput=False)
    minvT_d = nc.declare_dram_parameter("minvT", [S, SH], mybir.dt.uint16, isOutput=False)
    wqT_d = nc.declare_dram_parameter("wqT", [D, D], BF16, isOutput=False)
    wkT_d = nc.declare_dram_parameter("wkT", [D, D], BF16, isOutput=False)
    wvT_d = nc.declare_dram_parameter("wvT", [D, D], BF16, isOutput=False)
    woT_d = nc.declare_dram_parameter("woT", [D, D], BF16, isOutput=False)
    bq_d = nc.declare_dram_parameter("bq", [D, 1], F32, isOutput=False)
    bk_d = nc.declare_dram_parameter("bk", [D, 1], F32, isOutput=False)
    bv_d = nc.declare_dram_parameter("bv", [P, D], F32, isOutput=False)
    bo_d = nc.declare_dram_parameter("bo", [D, 1], F32, isOutput=False)
    sel_d = nc.declare_dram_parameter("sel", [NROW, NPAIR * SQB * P], F32, isOutput=False)
    out_d = nc.declare_dram_parameter("out", [D, SH], F32, isOutput=True)

    with tile.TileContext(nc) as tc:
        with (
            tc.tile_pool(name="persist", bufs=1) as persist,
            tc.tile_pool(name="psS", bufs=2, space="PSUM") as psS,
            tc.tile_pool(name="psV", bufs=4, space="PSUM") as psV,
        ):
            # ---- persistent SBUF: projection outputs + attention state ----
            qT = persist.tile([P, KC * SH], BF16)    # [D, SH]; d-chunk mc at mc*SH
            kT = persist.tile([P, KC * S], BF16)     # [D, S];  d-chunk mc at mc*S
            vpk = persist.tile([P, SKC * VROW], BF16)  # packed v+ones per sk chunk
            outMT = persist.tile([P, KC * SH], BF16)   # [D, SH] attn out^T, unnorm
            sums_pk = persist.tile([NROW, NB], F32)
            recip_pk = persist.tile([NROW, NB], F32)

            ones_nb = persist.tile([P, 2 * NB], BF16)
            nc.any.memset(ones_nb[:], 1.0)
            # selector for recip broadcast: sel[:, r*DH:(r+1)*DH] is one-hot
            # (row r all-ones); bcast_r = sel_r.T @ recip_pk
            sel = persist.tile([NROW, NPAIR * SQB * P], F32)
            nc.sync.dma_start(sel[:], sel_d[:])

            # ---------------- projections ----------------
            with tc.tile_pool(name="proj", bufs=1) as proj:
                qTb = proj.tile([P, KC * S], BF16)   # rotated query^T
                for kc in range(KC):
                    nc.sync.dma_start(qTb[:, kc * S:(kc + 1) * S],
                                      qT_d[kc * P:(kc + 1) * P, :])
                wq = proj.tile([P, KC * D], BF16)
                wk = proj.tile([P, KC * D], BF16)
                wv = proj.tile([P, KC * D], BF16)
                for kc in range(KC):
                    nc.sync.dma_start(wq[:, kc * D:(kc + 1) * D], wqT_d[kc * P:(kc + 1) * P, :])
                    nc.sync.dma_start(wk[:, kc * D:(kc + 1) * D], wkT_d[kc * P:(kc + 1) * P, :])
                    nc.sync.dma_start(wv[:, kc * D:(kc + 1) * D], wvT_d[kc * P:(kc + 1) * P, :])
                bq_sb = proj.tile([P, KC], F32)
                bk_sb = proj.tile([P, KC], F32)
                for mc in range(KC):
                    nc.sync.dma_start(bq_sb[:, mc:mc + 1], bq_d[mc * P:(mc + 1) * P, :])
                    nc.sync.dma_start(bk_sb[:, mc:mc + 1], bk_d[mc * P:(mc + 1) * P, :])
                bv_sb = proj.tile([P, D], F32)
                nc.sync.dma_start(bv_sb[:], bv_d[:])

                # qT [D, SH] (own half = rotated cols 0:SH) and kT [D, S]
                for dst, w, b_sb, ncols in ((qT, wq, bq_sb, SH), (kT, wk, bk_sb, S)):
                    for mc in range(KC):
                        for nb in range(ncols // NB):
                            ps = psS.tile([P, NB], F32, tag="sps")
                            for kc in range(KC):
                                nc.tensor.matmul(
                                    ps[:],
                                    w[:, kc * D + mc * P: kc * D + (mc + 1) * P],
                                    qTb[:, kc * S + nb * NB: kc * S + nb * NB + NB],
                                    start=(kc == 0), stop=(kc == KC - 1),
                                )
                            nc.scalar.activation(
                                dst[:, mc * ncols + nb * NB: mc * ncols + nb * NB + NB],
                                ps[:], AF.Identity, bias=b_sb[:, mc:mc + 1],
                            )

                # v_packed [S, H*VW]: sk-chunk sc at sc*VROW; head h at h*VW
                for sc in range(SKC):
                    for nb in range(2):  # heads 8*nb .. 8*nb+8
                        ps = psS.tile([P, NB], F32, tag="sps")
                        for kc in range(KC):
                            nc.tensor.matmul(
                                ps[:],
                                qTb[:, kc * S + sc * P: kc * S + (sc + 1) * P],
                                wv[:, kc * D + nb * NB: kc * D + (nb + 1) * NB],
                                start=(kc == 0), stop=(kc == KC - 1),
                            )
                        vdst3 = vpk[:, sc * VROW + nb * 8 * VW: sc * VROW + (nb * 8 + 8) * VW
                                    ].rearrange("p (h w) -> p h w", h=8)
                        nc.vector.tensor_add(
                            vdst3[:, :, 0:DH],
                            ps[:].rearrange("p (h w) -> p h w", h=8),
                            bv_sb[:, nb * NB:(nb + 1) * NB].rearrange("p (h w) -> p h w", h=8),
                        )
                        nc.any.memset(vdst3[:, :, DH:VW], 1.0)

            # ---------------- attention ----------------
            with (
                tc.tile_pool(name="mpool", bufs=1) as mpool,
                tc.tile_pool(name="apool", bufs=2) as apool,
                tc.tile_pool(name="work", bufs=2) as work,
            ):
                wo = work.tile([P, KC * D], BF16, tag="wo", bufs=1)
                for kc in range(KC):
                    nc.sync.dma_start(wo[:, kc * D:(kc + 1) * D], woT_d[kc * P:(kc + 1) * P, :])
                bo_sb = work.tile([P, KC], F32, tag="bo", bufs=1)
                for mc in range(KC):
                    nc.sync.dma_start(bo_sb[:, mc:mc + 1], bo_d[mc * P:(mc + 1) * P, :])

                for sq in range(SQB):
                    mT = mpool.tile([P, SKC * 2 * NB], mybir.dt.uint16, tag="mT")
                    for sc in range(SKC):
                        for dup in range(2):
                            nc.sync.dma_start(
                                mT[:, (2 * sc + dup) * NB:(2 * sc + dup + 1) * NB],
                                minvT_d[sc * P:(sc + 1) * P, sq * NB:(sq + 1) * NB],
                            )

                    for pr in range(NPAIR):
                        qs = pr * SH + sq * NB  # base col into qT chunk pr
                        pv0 = psV.tile([P, NB], F32, tag="psV")
                        pv1 = psV.tile([P, NB], F32, tag="psV")
                        for skh in range(2):  # two half-passes over sk chunks
                            a01 = apool.tile([P, SKH * 2 * NB], BF16, tag="a01")
                            for sc8 in range(SKH):
                                sc = skh * SKH + sc8
                                ks = pr * S + sc * P
                                sps = psS.tile([P, 2 * NB], F32, tag="sps")
                                nc.tensor.matmul(
                                    sps[:, 0:NB], kT[0:DH, ks:ks + P], qT[0:DH, qs:qs + NB],
                                    start=True, stop=True, tile_position=(0, 0),
                                )
                                nc.tensor.matmul(
                                    sps[:, NB:2 * NB], kT[DH:P, ks:ks + P], qT[DH:P, qs:qs + NB],
                                    start=True, stop=True, tile_position=(64, 0),
                                )
                                asl = a01[:, sc8 * 2 * NB:(sc8 + 1) * 2 * NB]
                                nc.scalar.activation(asl, sps[:], AF.Exp, scale=0.125)
                                nc.vector.copy_predicated(
                                    asl, mT[:, sc * 2 * NB:(sc + 1) * 2 * NB], ones_nb[:])
                            for h01, pv in ((0, pv0), (1, pv1)):
                                hloc = 2 * pr + h01
                                for sc8 in range(SKH):
                                    sc = skh * SKH + sc8
                                    nc.tensor.matmul(
                                        pv[0:VW, :],
                                        vpk[:, sc * VROW + hloc * VW: sc * VROW + (hloc + 1) * VW],
                                        a01[:, sc8 * 2 * NB + h01 * NB: sc8 * 2 * NB + (h01 + 1) * NB],
                                        start=(sc == 0), stop=(sc == SKC - 1),
                                    )
                        for h01, pv in ((0, pv0), (1, pv1)):
                            hloc = 2 * pr + h01
                            r = hloc * SQB + sq
                            # evac sums row (same partition), then DMA to its
                            # packed partition (DMA has no alignment rule)
                            sstage = work.tile([P, NB], F32, tag="sstage", bufs=3)
                            nc.vector.tensor_copy(sstage[DH:VW, :], pv[DH:VW, :])
                            nc.sync.dma_start(sums_pk[r:r + 1, :], sstage[DH:VW, :])
                            # outMT d-row block for head hloc: chunk pc = hloc//2,
                            # partition half h01
                            od = pr * SH + sq * NB
                            nc.vector.tensor_copy(
                                outMT[h01 * DH:(h01 + 1) * DH, od:od + NB],
                                pv[0:DH, :],
                            )

                # ---------------- softmax normalization ----------------
                nc.vector.reciprocal(recip_pk[:], sums_pk[:])
                for pr in range(NPAIR):
                    for sq in range(SQB):
                        blk = (pr * SQB + sq) * P
                        bc = psS.tile([P, NB], F32, tag="sps")
                        nc.tensor.matmul(bc[:], sel[:, blk:blk + P],
                                         recip_pk[:], start=True, stop=True)
                        od = pr * SH + sq * NB
                        osl = outMT[:, od:od + NB]
                        nc.vector.tensor_mul(osl, osl, bc[:])

                # ---------------- output projection ----------------
                for mc in range(KC):
                    for nb in range(SQB):
                        ps = psS.tile([P, NB], F32, tag="sps")
                        for kc in range(KC):
                            nc.tensor.matmul(
                                ps[:],
                                wo[:, kc * D + mc * P: kc * D + (mc + 1) * P],
                                outMT[:, kc * SH + nb * NB: kc * SH + nb * NB + NB],
                                start=(kc == 0), stop=(kc == KC - 1),
                            )
                        fin = work.tile([P, NB], F32, tag="fin", bufs=3)
                        nc.scalar.activation(fin[:], ps[:], AF.Identity, bias=bo_sb[:, mc:mc + 1])
                        nc.sync.dma_start(
                            out_d[mc * P:(mc + 1) * P, nb * NB:(nb + 1) * NB], fin[:])

    nc.finalize()
    return nc


_NC_CACHE = None
LAST_RESULTS = None


def _get_nc():
    global _NC_CACHE
    if _NC_CACHE is None:
        _NC_CACHE = _build_bass()
    return _NC_CACHE


def kernel(query, mask, Wq, bq, Wk, bk, Wv, bv, Wo, bo, **_unused):
    query = np.asarray(query, dtype=np.float32)
    mask = np.asarray(mask).astype(bool)
    Wq = np.asarray(Wq, dtype=np.float32)
    Wk = np.asarray(Wk, dtype=np.float32)
    Wv = np.asarray(Wv, dtype=np.float32)
    Wo = np.asarray(Wo, dtype=np.float32)
    bq = np.asarray(bq, dtype=np.float32)
    bk = np.asarray(bk, dtype=np.float32)
    bv = np.asarray(bv, dtype=np.float32)
    bo = np.asarray(bo, dtype=np.float32)

    wqT = np.ascontiguousarray(Wq.T).astype(_bf16)
    wkT = np.ascontiguousarray(Wk.T).astype(_bf16)
    wvT = np.ascontiguousarray(Wv.T).astype(_bf16)
    woT = np.ascontiguousarray(Wo.T).astype(_bf16)
    bq_c = np.ascontiguousarray(bq.reshape(D, 1))
    bk_c = np.ascontiguousarray(bk.reshape(D, 1))
    bv_x = np.ascontiguousarray(np.broadcast_to(bv, (P, D)))
    bo_c = np.ascontiguousarray(bo.reshape(D, 1))
    sel_np = np.zeros((NROW, NPAIR * SQB * P), dtype=np.float32)
    for pr in range(NPAIR):
        for sq in range(SQB):
            blk = (pr * SQB + sq) * P
            sel_np[(2 * pr) * SQB + sq, blk:blk + DH] = 1.0
            sel_np[(2 * pr + 1) * SQB + sq, blk + DH:blk + P] = 1.0

    in_maps = []
    for c in range(NCORES):
        b, half = c // 2, c % 2
        off = half * SH
        # rotate sequence so this core's half sits at columns [0, SH)
        qT_rot = np.ascontiguousarray(np.roll(query[b].T, -off, axis=1)).astype(_bf16)
        minv = (~mask[b]).T                       # [sk, sq], True where masked
        minv = np.roll(minv, -off, axis=0)        # rotate sk to match kT/v order
        minvT = np.ascontiguousarray(minv[:, off:off + SH]).astype(np.uint16)
        in_maps.append({
            "qT": qT_rot,
            "minvT": minvT,
            "wqT": wqT, "wkT": wkT, "wvT": wvT, "woT": woT,
            "bq": bq_c, "bk": bk_c, "bv": bv_x, "bo": bo_c, "sel": sel_np,
            "out": np.zeros((D, SH), dtype=np.float32),
        })

    nc = _get_nc()
    res = run_bass_kernel_spmd(nc, in_maps, core_ids=list(range(NCORES)))
    global LAST_RESULTS
    LAST_RESULTS = res

    out = np.empty((B, S, D), dtype=np.float32)
    for c in range(NCORES):
        b, half = c // 2, c % 2
        out[b, half * SH:(half + 1) * SH, :] = res.results[c]["out"].T
    return out
